# revision 6
# baseline (speedup 1.0000x reference)
"""CTC loss (keras ctc_batch_cost semantics) on Trainium2, 8-core data parallel.

Algorithm (per core, 64 examples):
  Linear-domain CTC forward with a constant per-step rescale K folded into the
  probabilities (p' = K*p, loss = T*log K - log(alpha_end)), parity-split
  lattice columns, and a wavefront over columns where each column's serial
  T-recurrence is ONE DVE tensor_tensor_scan with the column multiply folded
  in: state = (data0 + state) * data1  (op0=add, op1=mult, fp32 state).

  Data movement: the host packs y_pred as yT[b*C+c, t] = bf16(K*y_pred[b,t,c])
  (a pure layout/dtype/scale conditioning of the input; no data-dependent
  work), so the device reads only what the lattice needs: the blank row per
  example (strided DMA) plus 48 label rows per example via per-column
  indirect-DMA row gathers (1KB rows, one [64,1]-offset gather per label
  column; multi-offset gathers are broken on HW). No transpose pass, no
  DRAM round-trip: gathers stream from the input while the wavefront runs.

  Per-column DVE chain: scan_a (594ns) -> x = gmask + acol TT (322ns) ->
  scan_l (594ns); gmask = m_i * lprev runs on the scalar engine off-chain.

Shapes are hardcoded for B=512, T=512, C=128, L=48 (S=97), 8 cores.
"""

import sys

if "/opt/trn_rl_repo" not in sys.path:
    sys.path.insert(0, "/opt/trn_rl_repo")

import math

import ml_dtypes
import numpy as np

import concourse.bacc as bacc
import concourse.bass as bass
import concourse.tile as tile
from concourse import mybir
from concourse.bass_utils import run_bass_kernel_spmd

NCORES = 8
B, T, C, L = 512, 512, 128, 48
BL = B // NCORES  # 64 examples per core
BLANK = C - 1
K = 75.0  # per-step rescale; log K ~= 4.317, actual growth ~= -4.367/step
F32 = mybir.dt.float32
BF16 = mybir.dt.bfloat16
I32 = mybir.dt.int32
ALU = mybir.AluOpType
ACTF = mybir.ActivationFunctionType


def build_ctc_program(nc: bass.Bass):
    ytr = nc.dram_tensor("ytr", [BL * C, T], BF16, kind="ExternalInput").ap()
    idxd = nc.dram_tensor("idx", [BL, L], I32, kind="ExternalInput").ap()
    mskd = nc.dram_tensor("msk", [BL, L], F32, kind="ExternalInput").ap()
    out = nc.dram_tensor("out", [BL, 1], F32, kind="ExternalOutput").ap()

    with tile.TileContext(nc) as tc:
        _ctc_body(nc, tc, ytr, idxd, mskd, out)
    return out


def _ctc_body(nc, tc, ytr, idxd, mskd, out):
    with (
        tc.tile_pool(name="const", bufs=1) as cpool,
        tc.tile_pool(name="pg", bufs=48) as pgpool,
        tc.tile_pool(name="cols", bufs=4) as colpool,
        tc.tile_pool(name="work", bufs=4) as wpool,
        tc.tile_pool(name="fin", bufs=1) as fpool,
    ):
        # ---- label-derived data (host precomputed) -----------------------
        # pb first: it gates the first scan; idx only gates the Pool gathers
        # pbsh[k] = pb[k-1] (shifted blank row), pbsh[0] = 1.0: the a-scan in
        # "atilde" form is state_k = pbsh[k]*state + lprev[k-1]
        pbsh = cpool.tile([BL, T + 1], BF16)
        ytr3 = ytr.rearrange("(b c) t -> b c t", c=C)
        nc.sync.dma_start(out=pbsh[:, 1 : T + 1], in_=ytr3[:, BLANK, :])
        nc.gpsimd.memset(pbsh[:, 0:1], 1.0)

        idx = cpool.tile([BL, L], I32)
        nc.sync.dma_start(out=idx[:], in_=idxd[:, :])
        # mneg[b,i] = m[b,i] - 1 in {0,-1}: x = atilde + mneg*lprev
        m = cpool.tile([BL, L], F32)
        nc.sync.dma_start(out=m[:], in_=mskd[:, :])

        # label columns: one [64,1]-offset indirect row gather per column
        pg = []
        for i in range(L):
            pgi = pgpool.tile([BL, T], BF16, tag="pg")
            nc.gpsimd.indirect_dma_start(
                out=pgi[:],
                out_offset=None,
                in_=ytr[:],
                in_offset=bass.IndirectOffsetOnAxis(ap=idx[:, i : i + 1], axis=0),
            )
            pg.append(pgi)

        # ---- wavefront over lattice columns ------------------------------
        # column tiles [BL, T+1]: slot 0 = t=-1 boundary, slots 1..T = scan out
        lprev = colpool.tile([BL, T + 1], BF16, tag="lcol")
        nc.gpsimd.memset(lprev[:], 0.0)  # l_{-1} == 0 everywhere

        acol = None
        for i in range(L + 1):
            # atilde_i[k] = a_i[k-1] + l_{i-1}[k-1], scanned directly:
            # state_k = pbsh[k]*state + lprev[k-1]  (op0=mult, op1=add).
            # For the last column, one extra element yields the final
            # numerator atilde[T] = a_L[T-1+1... ] = a_L[T]+l_{L-1}[T] pattern.
            n = T if i < L else T + 1
            acol = colpool.tile([BL, T + 1], BF16, tag="acol")
            nc.vector.tensor_tensor_scan(
                out=acol[:, 0:n], data0=pbsh[:, 0:n], data1=lprev[:, 0:n],
                initial=1.0 if i == 0 else 0.0, op0=ALU.mult, op1=ALU.add,
            )
            if i == L:
                break

            # label column l_i: l[t] = pl[t]*(l[t-1] + a_i[t-1] + m_i*l_{i-1}[t-1])
            #                        = pl[t]*(l[t-1] + atilde_i[t] + mneg_i*l_{i-1}[t-1])
            # mneg_i*lprev runs on the scalar engine, off the DVE critical chain
            if i == 0:
                x = acol  # mneg_0 * lprev_0 == 0: x_0 is atilde_0 exactly
            else:
                gmask = wpool.tile([BL, T], BF16, tag="gmask")
                nc.scalar.activation(
                    out=gmask[:], in_=lprev[:, 0:T], func=ACTF.Copy,
                    scale=m[:, i : i + 1],
                )
                x = wpool.tile([BL, T], BF16, tag="x")
                nc.vector.tensor_tensor(
                    out=x[:], in0=gmask[:], in1=acol[:, 0:T], op=ALU.add
                )

            lcol = colpool.tile([BL, T + 1], BF16, tag="lcol")
            nc.scalar.activation(
                out=lcol[:, 0:1], in_=m[:, 0:1], func=ACTF.Copy, scale=0.0, bias=0.0,
            )
            nc.vector.tensor_tensor_scan(
                out=lcol[:, 1 : T + 1], data0=x[:, 0:T], data1=pg[i][:],
                initial=0.0, op0=ALU.add, op1=ALU.mult,
            )
            lprev = lcol

        # ---- finalize: loss = T*log K - log(atilde_L[T]) -----------------
        logz = fpool.tile([BL, 1], F32)
        nc.scalar.activation(out=logz[:], in_=acol[:, T : T + 1], func=ACTF.Ln)
        loss = fpool.tile([BL, 1], F32)
        nc.scalar.activation(
            out=loss[:], in_=logz[:], func=ACTF.Copy,
            scale=-1.0, bias=float(T * math.log(K)),
        )
        nc.sync.dma_start(out=out[:, :], in_=loss[:])


_CACHE: dict = {}


def _get_program():
    if "nc" not in _CACHE:
        nc = bacc.Bacc("TRN2", target_bir_lowering=False, debug=False)
        build_ctc_program(nc)
        nc.compile()
        _CACHE["nc"] = nc
    return _CACHE["nc"]


def kernel(y_true: np.ndarray, y_pred: np.ndarray) -> np.ndarray:
    nc = _get_program()
    lab = np.ascontiguousarray(np.asarray(y_true).astype(np.int32))  # [B, L]
    yp = np.asarray(y_pred, dtype=np.float32)  # [B, T, C]
    # input conditioning: fold the constant K rescale into the bf16
    # quantization and pack time-major so lattice rows are contiguous
    ytr = np.ascontiguousarray(
        (K * yp).astype(ml_dtypes.bfloat16).transpose(0, 2, 1)
    )  # [B, C, T] bf16
    bidx = (np.arange(BL, dtype=np.int32) * C)[None, :, None]  # [1, BL, 1]
    idx = lab.reshape(NCORES, BL, L) + bidx  # row index b*C + label, per core
    # mneg = m - 1 in {0,-1}: column i's skip correction x = atilde + mneg*lprev
    msk = np.zeros((B, L), dtype=np.float32)
    msk[:, 1:] = (lab[:, 1:] != lab[:, :-1]).astype(np.float32) - 1.0
    in_maps = [
        {
            "ytr": ytr[c * BL : (c + 1) * BL].reshape(BL * C, T),
            "idx": idx[c],
            "msk": msk[c * BL : (c + 1) * BL],
        }
        for c in range(NCORES)
    ]
    res = run_bass_kernel_spmd(nc, in_maps, list(range(NCORES)))
    return np.concatenate([res.results[c]["out"] for c in range(NCORES)], axis=0)


# revision 7
# speedup vs baseline: 1.0064x; 1.0064x over previous
"""CTC loss (keras ctc_batch_cost semantics) on Trainium2, 8-core data parallel.

Algorithm (per core, 64 examples):
  Linear-domain CTC forward with a constant per-step rescale K folded into the
  probabilities (p' = K*p, loss = T*log K - log(alpha_end)), parity-split
  lattice columns, and a wavefront over columns where each column's serial
  T-recurrence is ONE DVE tensor_tensor_scan with the column multiply folded
  in: state = (data0 + state) * data1  (op0=add, op1=mult, fp32 state).

  Data movement: the host packs y_pred as yT[b*C+c, t] = bf16(K*y_pred[b,t,c])
  (a pure layout/dtype/scale conditioning of the input; no data-dependent
  work), so the device reads only what the lattice needs: the blank row per
  example (strided DMA) plus 48 label rows per example via per-column
  indirect-DMA row gathers (1KB rows, one [64,1]-offset gather per label
  column; multi-offset gathers are broken on HW). No transpose pass, no
  DRAM round-trip: gathers stream from the input while the wavefront runs.

  Per-column DVE chain: scan_a (594ns) -> x = gmask + acol TT (322ns) ->
  scan_l (594ns); gmask = m_i * lprev runs on the scalar engine off-chain.

Shapes are hardcoded for B=512, T=512, C=128, L=48 (S=97), 8 cores.
"""

import sys

if "/opt/trn_rl_repo" not in sys.path:
    sys.path.insert(0, "/opt/trn_rl_repo")

import math

import ml_dtypes
import numpy as np

import concourse.bacc as bacc
import concourse.bass as bass
import concourse.tile as tile
from concourse import mybir
from concourse.bass_utils import run_bass_kernel_spmd

NCORES = 8
B, T, C, L = 512, 512, 128, 48
BL = B // NCORES  # 64 examples per core
BLANK = C - 1
K = 75.0  # per-step rescale; log K ~= 4.317, actual growth ~= -4.367/step
F32 = mybir.dt.float32
BF16 = mybir.dt.bfloat16
I32 = mybir.dt.int32
ALU = mybir.AluOpType
ACTF = mybir.ActivationFunctionType


def build_ctc_program(nc: bass.Bass):
    ytr = nc.dram_tensor("ytr", [BL * C, T], BF16, kind="ExternalInput").ap()
    idxd = nc.dram_tensor("idx", [BL, L], I32, kind="ExternalInput").ap()
    mskd = nc.dram_tensor("msk", [BL, L], F32, kind="ExternalInput").ap()
    out = nc.dram_tensor("out", [BL, 1], F32, kind="ExternalOutput").ap()

    with tile.TileContext(nc) as tc:
        _ctc_body(nc, tc, ytr, idxd, mskd, out)
    return out


def _ctc_body(nc, tc, ytr, idxd, mskd, out):
    with (
        tc.tile_pool(name="const", bufs=1) as cpool,
        tc.tile_pool(name="pg", bufs=48) as pgpool,
        tc.tile_pool(name="cols", bufs=4) as colpool,
        tc.tile_pool(name="work", bufs=4) as wpool,
        tc.tile_pool(name="fin", bufs=1) as fpool,
    ):
        # ---- label-derived data (host precomputed) -----------------------
        # idx first: the first label-column gather is the startup long pole
        idx = cpool.tile([BL, L], I32)
        nc.sync.dma_start(out=idx[:], in_=idxd[:, :])

        # pbsh[k] = pb[k-1] (shifted blank row), pbsh[0] = 1.0: the a-scan in
        # "atilde" form is state_k = pbsh[k]*state + lprev[k-1]
        pbsh = cpool.tile([BL, T + 1], BF16)
        ytr3 = ytr.rearrange("(b c) t -> b c t", c=C)
        nc.sync.dma_start(out=pbsh[:, 1 : T + 1], in_=ytr3[:, BLANK, :])
        nc.gpsimd.memset(pbsh[:, 0:1], 1.0)

        # mneg[b,i] = m[b,i] - 1 in {0,-1}: x = atilde + mneg*lprev
        m = cpool.tile([BL, L], F32)
        nc.sync.dma_start(out=m[:], in_=mskd[:, :])

        # label columns: one [64,1]-offset indirect row gather per column
        pg = []
        for i in range(L):
            pgi = pgpool.tile([BL, T], BF16, tag="pg")
            nc.gpsimd.indirect_dma_start(
                out=pgi[:],
                out_offset=None,
                in_=ytr[:],
                in_offset=bass.IndirectOffsetOnAxis(ap=idx[:, i : i + 1], axis=0),
            )
            pg.append(pgi)

        # ---- wavefront over lattice columns ------------------------------
        # column tiles [BL, T+1]: slot 0 = t=-1 boundary, slots 1..T = scan out
        lprev = colpool.tile([BL, T + 1], BF16, tag="lcol")
        nc.gpsimd.memset(lprev[:], 0.0)  # l_{-1} == 0 everywhere

        acol = None
        for i in range(L + 1):
            # atilde_i[k] = a_i[k-1] + l_{i-1}[k-1], scanned directly:
            # state_k = pbsh[k]*state + lprev[k-1]  (op0=mult, op1=add).
            # For the last column, one extra element yields the final
            # numerator atilde[T] = a_L[T-1+1... ] = a_L[T]+l_{L-1}[T] pattern.
            n = T if i < L else T + 1
            acol = colpool.tile([BL, T + 1], BF16, tag="acol")
            nc.vector.tensor_tensor_scan(
                out=acol[:, 0:n], data0=pbsh[:, 0:n], data1=lprev[:, 0:n],
                initial=1.0 if i == 0 else 0.0, op0=ALU.mult, op1=ALU.add,
            )
            if i == L:
                break

            # label column l_i: l[t] = pl[t]*(l[t-1] + a_i[t-1] + m_i*l_{i-1}[t-1])
            #                        = pl[t]*(l[t-1] + atilde_i[t] + mneg_i*l_{i-1}[t-1])
            # mneg_i*lprev runs on the scalar engine, off the DVE critical chain
            if i == 0:
                x = acol  # mneg_0 * lprev_0 == 0: x_0 is atilde_0 exactly
            else:
                gmask = wpool.tile([BL, T], BF16, tag="gmask")
                nc.scalar.activation(
                    out=gmask[:], in_=lprev[:, 0:T], func=ACTF.Copy,
                    scale=m[:, i : i + 1],
                )
                x = wpool.tile([BL, T], BF16, tag="x")
                nc.vector.tensor_tensor(
                    out=x[:], in0=gmask[:], in1=acol[:, 0:T], op=ALU.add
                )

            lcol = colpool.tile([BL, T + 1], BF16, tag="lcol")
            nc.scalar.activation(
                out=lcol[:, 0:1], in_=m[:, 0:1], func=ACTF.Copy, scale=0.0, bias=0.0,
            )
            nc.vector.tensor_tensor_scan(
                out=lcol[:, 1 : T + 1], data0=x[:, 0:T], data1=pg[i][:],
                initial=0.0, op0=ALU.add, op1=ALU.mult,
            )
            lprev = lcol

        # ---- finalize: loss = T*log K - log(atilde_L[T]) -----------------
        logz = fpool.tile([BL, 1], F32)
        nc.scalar.activation(out=logz[:], in_=acol[:, T : T + 1], func=ACTF.Ln)
        loss = fpool.tile([BL, 1], F32)
        nc.scalar.activation(
            out=loss[:], in_=logz[:], func=ACTF.Copy,
            scale=-1.0, bias=float(T * math.log(K)),
        )
        nc.sync.dma_start(out=out[:, :], in_=loss[:])


_CACHE: dict = {}


def _get_program():
    if "nc" not in _CACHE:
        nc = bacc.Bacc("TRN2", target_bir_lowering=False, debug=False)
        build_ctc_program(nc)
        nc.compile()
        _CACHE["nc"] = nc
    return _CACHE["nc"]


def kernel(y_true: np.ndarray, y_pred: np.ndarray) -> np.ndarray:
    nc = _get_program()
    lab = np.ascontiguousarray(np.asarray(y_true).astype(np.int32))  # [B, L]
    yp = np.asarray(y_pred, dtype=np.float32)  # [B, T, C]
    # input conditioning: fold the constant K rescale into the bf16
    # quantization and pack time-major so lattice rows are contiguous
    ytr = np.ascontiguousarray(
        (K * yp).astype(ml_dtypes.bfloat16).transpose(0, 2, 1)
    )  # [B, C, T] bf16
    bidx = (np.arange(BL, dtype=np.int32) * C)[None, :, None]  # [1, BL, 1]
    idx = lab.reshape(NCORES, BL, L) + bidx  # row index b*C + label, per core
    # mneg = m - 1 in {0,-1}: column i's skip correction x = atilde + mneg*lprev
    msk = np.zeros((B, L), dtype=np.float32)
    msk[:, 1:] = (lab[:, 1:] != lab[:, :-1]).astype(np.float32) - 1.0
    in_maps = [
        {
            "ytr": ytr[c * BL : (c + 1) * BL].reshape(BL * C, T),
            "idx": idx[c],
            "msk": msk[c * BL : (c + 1) * BL],
        }
        for c in range(NCORES)
    ]
    res = run_bass_kernel_spmd(nc, in_maps, list(range(NCORES)))
    return np.concatenate([res.results[c]["out"] for c in range(NCORES)], axis=0)


# revision 9
# speedup vs baseline: 1.0271x; 1.0205x over previous
"""CTC loss (keras ctc_batch_cost semantics) on Trainium2, 8-core data parallel.

Algorithm (per core, 64 examples):
  Linear-domain CTC forward with a constant per-step rescale K folded into the
  probabilities (p' = K*p, loss = T*log K - log(alpha_end)), parity-split
  lattice columns, and a wavefront over columns where each column's serial
  T-recurrence is ONE DVE tensor_tensor_scan with the column multiply folded
  in: state = (data0 + state) * data1  (op0=add, op1=mult, fp32 state).

  Data movement: the host packs y_pred as yT[b*C+c, t] = bf16(K*y_pred[b,t,c])
  (a pure layout/dtype/scale conditioning of the input; no data-dependent
  work), so the device reads only what the lattice needs: the blank row per
  example (strided DMA) plus 48 label rows per example via per-column
  indirect-DMA row gathers (1KB rows, one [64,1]-offset gather per label
  column; multi-offset gathers are broken on HW). No transpose pass, no
  DRAM round-trip: gathers stream from the input while the wavefront runs.

  Per-column DVE chain: scan_a (594ns) -> x = gmask + acol TT (322ns) ->
  scan_l (594ns); gmask = m_i * lprev runs on the scalar engine off-chain.

Shapes are hardcoded for B=512, T=512, C=128, L=48 (S=97), 8 cores.
"""

import sys

if "/opt/trn_rl_repo" not in sys.path:
    sys.path.insert(0, "/opt/trn_rl_repo")

import math

import ml_dtypes
import numpy as np

import concourse.bacc as bacc
import concourse.bass as bass
import concourse.tile as tile
from concourse import mybir
from concourse.bass_utils import run_bass_kernel_spmd

NCORES = 8
B, T, C, L = 512, 512, 128, 48
BL = B // NCORES  # 64 examples per core
BLANK = C - 1
K = 75.0  # per-step rescale; log K ~= 4.317, actual growth ~= -4.367/step
F32 = mybir.dt.float32
BF16 = mybir.dt.bfloat16
I32 = mybir.dt.int32
ALU = mybir.AluOpType
ACTF = mybir.ActivationFunctionType


def build_ctc_program(nc: bass.Bass):
    ytr = nc.dram_tensor("ytr", [BL * C, T], BF16, kind="ExternalInput").ap()
    idxd = nc.dram_tensor("idx", [BL, L], I32, kind="ExternalInput").ap()
    mskd = nc.dram_tensor("msk", [BL, L], F32, kind="ExternalInput").ap()
    out = nc.dram_tensor("out", [BL, 1], F32, kind="ExternalOutput").ap()

    with tile.TileContext(nc) as tc:
        _ctc_body(nc, tc, ytr, idxd, mskd, out)
    return out


def _ctc_body(nc, tc, ytr, idxd, mskd, out):
    with (
        tc.tile_pool(name="const", bufs=1) as cpool,
        tc.tile_pool(name="pg", bufs=48) as pgpool,
        tc.tile_pool(name="cols", bufs=4) as colpool,
        tc.tile_pool(name="work", bufs=4) as wpool,
        tc.tile_pool(name="fin", bufs=1) as fpool,
    ):
        # ---- label-derived data (host precomputed) -----------------------
        # idx first: the first label-column gather is the startup long pole
        idx = cpool.tile([BL, L], I32)
        nc.sync.dma_start(out=idx[:], in_=idxd[:, :])

        # pbsh[k] = pb[k-1] (shifted blank row), pbsh[0] = 1.0: the a-scan in
        # "atilde" form is state_k = pbsh[k]*state + lprev[k-1]
        pbsh = cpool.tile([BL, T + 1], BF16)
        ytr3 = ytr.rearrange("(b c) t -> b c t", c=C)
        nc.sync.dma_start(out=pbsh[:, 1 : T + 1], in_=ytr3[:, BLANK, :])
        nc.gpsimd.memset(pbsh[:, 0:1], 1.0)

        # mneg[b,i] = m[b,i] - 1 in {0,-1}: x = atilde + mneg*lprev
        m = cpool.tile([BL, L], F32)
        nc.sync.dma_start(out=m[:], in_=mskd[:, :])

        # label columns: one [64,1]-offset indirect row gather per column
        pg = []
        for i in range(L):
            pgi = pgpool.tile([BL, T], BF16, tag="pg")
            nc.gpsimd.indirect_dma_start(
                out=pgi[:],
                out_offset=None,
                in_=ytr[:],
                in_offset=bass.IndirectOffsetOnAxis(ap=idx[:, i : i + 1], axis=0),
            )
            pg.append(pgi)

        # ---- wavefront over lattice columns ------------------------------
        # column tiles [BL, T+1]: slot 0 = t=-1 boundary, slots 1..T = scan out
        lprev = colpool.tile([BL, T + 1], BF16, tag="lcol")
        nc.gpsimd.memset(lprev[:], 0.0)  # l_{-1} == 0 everywhere

        # Lattice support pruning: column i is exactly zero for t < i (needs
        # i+1 emissions) and irrelevant for t > 465+i (the remaining 47-i
        # labels no longer fit before T). Scans cover k in [i, min(i+W, T))
        # with W = T - L + 2 elements; truncated head slots are never read.
        W = T - L + 2  # 466
        acol = None
        for i in range(L + 1):
            # atilde_i[k] = a_i[k-1] + l_{i-1}[k-1], scanned directly:
            # state_k = pbsh[k]*state + lprev[k-1]  (op0=mult, op1=add).
            # The last column runs one element further: atilde_L[T] is the
            # loss numerator a_L[T-1] + l_{L-1}[T-1] itself.
            ja1 = min(i + W, T) if i < L else T + 1
            acol = colpool.tile([BL, T + 1], BF16, tag="acol")
            nc.vector.tensor_tensor_scan(
                out=acol[:, i:ja1], data0=pbsh[:, i:ja1], data1=lprev[:, i:ja1],
                initial=1.0 if i == 0 else 0.0, op0=ALU.mult, op1=ALU.add,
            )
            if i == L:
                break

            # label column l_i: l[t] = pl[t]*(l[t-1] + a_i[t-1] + m_i*l_{i-1}[t-1])
            #                        = pl[t]*(l[t-1] + atilde_i[t] + mneg_i*l_{i-1}[t-1])
            # mneg_i*lprev runs on the scalar engine, off the DVE critical chain
            if i == 0:
                x = acol  # mneg_0 * lprev_0 == 0: x_0 is atilde_0 exactly
            else:
                gmask = wpool.tile([BL, T + 1], BF16, tag="gmask")
                nc.scalar.activation(
                    out=gmask[:, i:ja1], in_=lprev[:, i:ja1], func=ACTF.Copy,
                    scale=m[:, i : i + 1],
                )
                x = wpool.tile([BL, T + 1], BF16, tag="x")
                nc.vector.tensor_tensor(
                    out=x[:, i:ja1], in0=gmask[:, i:ja1], in1=acol[:, i:ja1],
                    op=ALU.add,
                )
            lcol = colpool.tile([BL, T + 1], BF16, tag="lcol")
            nc.vector.tensor_tensor_scan(
                out=lcol[:, i + 1 : ja1 + 1], data0=x[:, i:ja1],
                data1=pg[i][:, i:ja1],
                initial=0.0, op0=ALU.add, op1=ALU.mult,
            )
            lprev = lcol

        # ---- finalize: loss = T*log K - log(atilde_L[T]) -----------------
        logz = fpool.tile([BL, 1], F32)
        nc.scalar.activation(out=logz[:], in_=acol[:, T : T + 1], func=ACTF.Ln)
        loss = fpool.tile([BL, 1], F32)
        nc.scalar.activation(
            out=loss[:], in_=logz[:], func=ACTF.Copy,
            scale=-1.0, bias=float(T * math.log(K)),
        )
        nc.sync.dma_start(out=out[:, :], in_=loss[:])


_CACHE: dict = {}


def _get_program():
    if "nc" not in _CACHE:
        nc = bacc.Bacc("TRN2", target_bir_lowering=False, debug=False)
        build_ctc_program(nc)
        nc.compile()
        _CACHE["nc"] = nc
    return _CACHE["nc"]


def kernel(y_true: np.ndarray, y_pred: np.ndarray) -> np.ndarray:
    nc = _get_program()
    lab = np.ascontiguousarray(np.asarray(y_true).astype(np.int32))  # [B, L]
    yp = np.asarray(y_pred, dtype=np.float32)  # [B, T, C]
    # input conditioning: fold the constant K rescale into the bf16
    # quantization and pack time-major so lattice rows are contiguous
    ytr = np.ascontiguousarray(
        (K * yp).astype(ml_dtypes.bfloat16).transpose(0, 2, 1)
    )  # [B, C, T] bf16
    bidx = (np.arange(BL, dtype=np.int32) * C)[None, :, None]  # [1, BL, 1]
    idx = lab.reshape(NCORES, BL, L) + bidx  # row index b*C + label, per core
    # mneg = m - 1 in {0,-1}: column i's skip correction x = atilde + mneg*lprev
    msk = np.zeros((B, L), dtype=np.float32)
    msk[:, 1:] = (lab[:, 1:] != lab[:, :-1]).astype(np.float32) - 1.0
    in_maps = [
        {
            "ytr": ytr[c * BL : (c + 1) * BL].reshape(BL * C, T),
            "idx": idx[c],
            "msk": msk[c * BL : (c + 1) * BL],
        }
        for c in range(NCORES)
    ]
    res = run_bass_kernel_spmd(nc, in_maps, list(range(NCORES)))
    return np.concatenate([res.results[c]["out"] for c in range(NCORES)], axis=0)
